# revision 24
# baseline (speedup 1.0000x reference)
"""EquivariantUpdate Bass kernel for 8 TRN2 NeuronCores (v3).

Strategy (edge-sharded):
- Host: shard E=800k edges 8 ways; per core, bucket edges by
  (row>=25000, col>=25000) so dma_gather's int16 indices address
  half-table views; within a bucket sort by row and pack into 128-edge
  tiles, cutting a tile early when its rows would span >= W nodes past
  its 128-aligned window base (rare).
- Device phase 0: one combined table abtab[n] = [h@W1a.T | h@W1b.T]
  (bf16, [N, 256]), built in 256-node slabs (1 load + 2 matmuls +
  1 act-copy + 1 store per slab).
- Device phase 1 per 2048 edges: dma_gather A[row], B[col] with
  transpose=True + elem_step so features land on partitions ([f, e]
  layout); layer-1 pre = w1c (x) attr (rank-1 matmul) + identity-matmul
  copies of the two gathers accumulated in PSUM; silu; W2 matmul; silu;
  phi via 4 [x2-chunk]x[w3 x12] matmuls -> tr12 = phi12 * cdm12 (DVE,
  cdm12 is host-zero-expanded over the 4 window chunks); segment-sum:
  S = is_equal(iota128, row mod 128) one-hot (DVE) and ONE n=12 matmul
  per tile accumulating [128, 12] = [node-in-chunk, (chunk, xyz)];
  4 tiles share a [128, 48] PSUM block -> act-copy -> one DMA per
  block to a per-block DRAM slot (no indirect DMA, no scatter).
- Host: combine per-tile blocks into agg (np.add.at over node tiles),
  sum over cores, out = (coord + agg) * node_mask.
  (1/NORM_FACTOR folded into W3; edge_mask folded into coord_diff.)
"""
import contextlib

import numpy as np
import ml_dtypes

import concourse.bass as bass
import concourse.bacc as bacc
import concourse.mybir as mybir
import concourse.tile as tile
from concourse.bass_utils import run_bass_kernel_spmd
from concourse.masks import make_identity

P = 128
N = 50000
NPAD = 50176                 # N padded to 512 for uniform phase-0 slabs
H = 128
E = 800000
NCORES = 8
ECORE = E // NCORES          # 100000
HALF = 25000                 # table split point (halves stay int16-safe)
W = 512                      # segsum window (nodes per tile span cap)
WC = W // 128                # 4 chunks
NI = 6656                    # indices per dma_gather instruction
TPG = NI // P                # 16 tiles per gather
TB = 208                     # tile slots per bucket
GI_PER_B = TB * P // NI      # 13 gathers per bucket
NBUCK = 4
TTOT = NBUCK * TB            # 832 tiles per core
NBLK = TTOT // 4             # 208 blocks of 512 edges

BF16 = mybir.dt.bfloat16
FP16 = mybir.dt.float16
F32 = mybir.dt.float32
I16 = mybir.dt.int16

_nc_cache = {}


def _wrap_idx(idx_flat):
    """int16 gather indices -> wrapped [16, NI/16] replicated to [128, NI/16]."""
    w = idx_flat.reshape(NI // 16, 16).T.astype(np.int16)
    return np.tile(w, (8, 1))


def _build_program(loop_k=0):
    import os
    ablate = os.environ.get("KABLATE", "")
    nc = bacc.Bacc(None, target_bir_lowering=False, num_swdge_queues=4)

    hT_t = nc.dram_tensor("hT", [H, NPAD], BF16, kind="ExternalInput")
    w1abT_t = nc.dram_tensor("w1abT", [H, 2 * H], BF16, kind="ExternalInput")
    w1c_t = nc.dram_tensor("w1c", [1, H], BF16, kind="ExternalInput")
    w2T_t = nc.dram_tensor("w2T", [H, H], BF16, kind="ExternalInput")
    w3x12_t = nc.dram_tensor("w3x12", [H, 12], BF16, kind="ExternalInput")
    b1_t = nc.dram_tensor("b1c", [H, 1], F32, kind="ExternalInput")
    b2_t = nc.dram_tensor("b2c", [H, 1], F32, kind="ExternalInput")
    idxg_t = nc.dram_tensor("idxg", [P, NBUCK * GI_PER_B * 2 * (NI // 16)],
                            I16, kind="ExternalInput")
    roff_t = nc.dram_tensor("roff", [P, TTOT], FP16, kind="ExternalInput")
    cdm12_t = nc.dram_tensor("cdm12", [P, TTOT * 12], BF16,
                             kind="ExternalInput")
    attrT_t = nc.dram_tensor("attrT", [NBUCK, 1, TB * P], BF16,
                             kind="ExternalInput")

    blk_t = nc.dram_tensor("blk", [NBLK // 4, P, 192], BF16,
                       kind="ExternalOutput")

    HPAD = NPAD // 2  # 25088
    ablo = nc.dram_tensor("ablo", [HPAD, 2 * H], BF16)
    abhi = nc.dram_tensor("abhi", [HPAD, 2 * H], BF16)

    with tile.TileContext(nc) as tc:
        with (
            tc.tile_pool(name="static", bufs=1) as stp,
            tc.tile_pool(name="p0", bufs=3) as p0p,
            tc.tile_pool(name="gat", bufs=2) as gap,
            tc.tile_pool(name="blk", bufs=4) as blp,
            tc.tile_pool(name="ps", bufs=2, space="PSUM") as psp,
            tc.tile_pool(name="p0ps", bufs=2, space="PSUM") as p0ps,
            tc.tile_pool(name="phiseg", bufs=2, space="PSUM") as phps,
            tc.tile_pool(name="sc", bufs=6) as scp,
        ):
            # ---- statics ----
            identf = stp.tile([P, P], F32, name="identf")
            make_identity(nc, identf[:])
            identB = stp.tile([P, P], BF16, name="identB")
            nc.vector.tensor_copy(identB[:], identf[:])
            w1abT = stp.tile([H, 2 * H], BF16, name="w1abT")
            nc.sync.dma_start(out=w1abT[:], in_=w1abT_t[:, :])
            w1c = stp.tile([1, H], BF16, name="w1c")
            nc.sync.dma_start(out=w1c[:], in_=w1c_t[:, :])
            w2T = stp.tile([H, H], BF16, name="w2T")
            nc.sync.dma_start(out=w2T[:], in_=w2T_t[:, :])
            w3x12 = stp.tile([H, 12], BF16, name="w3x12")
            nc.sync.dma_start(out=w3x12[:], in_=w3x12_t[:, :])
            b1 = stp.tile([H, 1], F32, name="b1")
            nc.sync.dma_start(out=b1[:], in_=b1_t[:, :])
            b2 = stp.tile([H, 1], F32, name="b2")
            nc.sync.dma_start(out=b2[:], in_=b2_t[:, :])
            iota4 = stp.tile([P, 4 * P], FP16, name="iota4")
            nc.gpsimd.iota(iota4[:].rearrange("p (t c) -> p t c", t=4),
                           pattern=[[0, 4], [1, P]], base=0,
                           channel_multiplier=0,
                           allow_small_or_imprecise_dtypes=True)
            rowoffs = stp.tile([P, TTOT], FP16, name="rowoffs")
            nc.sync.dma_start(out=rowoffs[:], in_=roff_t[:, :])
            cdm12R = stp.tile([P, TTOT * 12], BF16, name="cdm12R")
            nc.sync.dma_start(out=cdm12R[:], in_=cdm12_t[:, :])
            idxsb = stp.tile([P, NBUCK * GI_PER_B * 2 * (NI // 16)], I16,
                             name="idxsb")
            nc.sync.dma_start(out=idxsb[:], in_=idxg_t[:, :])
            idx_v = idxsb[:].rearrange("p (b g t w) -> b g t p w", b=NBUCK,
                                       g=GI_PER_B, t=2)

            loop_cm = tc.For_i(0, loop_k, 1) if loop_k else contextlib.nullcontext()
            loop_cm.__enter__()

            # ---- phase 0 helper: build one half-table ----
            def build_half(tab, hbase):
                NSS = (NPAD // 2) // 512  # 49
                for s in range(NSS if "nophase0" not in ablate else 0):
                    n0 = s * 512
                    ht = p0p.tile([H, 512], BF16, tag="ht", name="ht")
                    nc.sync.dma_start(out=ht[:],
                                      in_=hT_t[:, hbase + n0:hbase + n0 + 512])
                    absb2 = p0p.tile([P, 1024], BF16, tag="absb2",
                                     name="absb2")
                    for half in range(2):
                        ab = p0ps.tile([H, 512], F32, space="PSUM",
                                       tag="abps", name="abps")
                        for k in range(2):
                            nc.tensor.matmul(
                                ab[:, 256 * k:256 * k + 256],
                                lhsT=ht[:, 256 * half + k * P:
                                        256 * half + k * P + P],
                                rhs=w1abT[:], start=True, stop=True,
                                skip_group_check=True)
                        nc.vector.tensor_copy(
                            absb2[:, 512 * half:512 * half + 512], ab[:])
                    nc.sync.dma_start(
                        out=tab[n0:n0 + 512, :].rearrange(
                            "(k p) c -> p k c", p=P),
                        in_=absb2[:].rearrange("p (k c) -> p k c", k=4))

            # ---- phase 1: edges ----
            def run_bucket(b):
                atab_v = (abhi if b >= 2 else ablo)[:, :H]
                btab_v = (abhi if (b % 2) else ablo)[:, H:]
                attrT = stp.tile([1, TB * P], BF16, tag="attrT", name="attrT")
                nc.sync.dma_start(out=attrT[:], in_=attrT_t[b, :, :])
                for gi in range(GI_PER_B):
                    ga = gap.tile([P, NI], BF16, tag="ga", name="ga")
                    gb = gap.tile([P, NI], BF16, tag="gb", name="gb")
                    if "nogather" in ablate:
                        nc.vector.memset(ga[:, :1], 0.0)
                        nc.vector.memset(gb[:, :1], 0.0)
                    else:
                        nc.gpsimd.dma_gather(
                            out_ap=ga[:].rearrange("p (b n) -> p b n", n=NI),
                            in_ap=atab_v, idxs_ap=idx_v[b, gi, 0], num_idxs=NI,
                            num_idxs_reg=NI, elem_size=H, elem_step=2 * H,
                            transpose=True, single_packet=False,
                            queue_num=(2 * gi) % 4)
                        nc.gpsimd.dma_gather(
                            out_ap=gb[:].rearrange("p (b n) -> p b n", n=NI),
                            in_ap=btab_v, idxs_ap=idx_v[b, gi, 1], num_idxs=NI,
                            num_idxs_reg=NI, elem_size=H, elem_step=2 * H,
                            transpose=True, single_packet=False,
                            queue_num=(2 * gi + 1) % 4)

                    for bk in range(TPG // 4):     # blocks of 512 edges
                        t0 = gi * TPG + bk * 4     # tile within bucket
                        tg0 = b * TB + t0          # global tile
                        bg = tg0 // 4              # global block
                        cols = slice(bk * 4 * P, (bk + 1) * 4 * P)
                        e0 = t0 * P
                        pre = psp.tile([H, 512], F32, space="PSUM", tag="pre",
                                       name="pre")
                        nc.tensor.matmul(pre[:], lhsT=w1c[:],
                                         rhs=attrT[:, e0:e0 + 512],
                                         start=True, stop=False)
                        nc.tensor.matmul(pre[:], lhsT=identB[:],
                                         rhs=ga[:, cols], start=False,
                                         stop=False)
                        nc.tensor.matmul(pre[:], lhsT=identB[:],
                                         rhs=gb[:, cols], start=False,
                                         stop=True)
                        x1 = blp.tile([H, 512], BF16, tag="x1", name="x1")
                        nc.scalar.activation(
                            x1[:], pre[:],
                            mybir.ActivationFunctionType.Silu, bias=b1[:, :1])
                        pre2 = psp.tile([H, 512], F32, space="PSUM", tag="pre2",
                                        name="pre2")
                        nc.tensor.matmul(pre2[:], lhsT=w2T[:], rhs=x1[:],
                                         start=True, stop=True)
                        x2 = blp.tile([H, 512], BF16, tag="x2", name="x2")
                        nc.scalar.activation(
                            x2[:], pre2[:],
                            mybir.ActivationFunctionType.Silu, bias=b2[:, :1])
                        phiseg = phps.tile([P, 96], F32, space="PSUM",
                                           tag="phiseg", name="phiseg")
                        phips = phiseg[:, 0:48]
                        segps = phiseg[:, 48:96]
                        for k in range(4):
                            nc.tensor.matmul(
                                phiseg[:, 12 * k:12 * k + 12],
                                lhsT=x2[:, k * P:(k + 1) * P],
                                rhs=w3x12[:], start=True, stop=True,
                                skip_group_check=True)
                        tr12 = scp.tile([P, 48], FP16, tag="tr12", name="tr12")
                        nc.vector.tensor_mul(
                            tr12[:], phips,
                            cdm12R[:, 12 * tg0:12 * tg0 + 48])
                        if "noseg" not in ablate:
                            S4 = scp.tile([P, 4 * P], FP16, tag="S4",
                                          name="S4")
                            i0b, i1b = bass.broadcast_tensor_aps(
                                iota4[:].rearrange("p (t c) -> p t c", t=4),
                                rowoffs[:, tg0:tg0 + 4].rearrange(
                                    "p (t one) -> p t one", one=1))
                            nc.vector.tensor_tensor(
                                out=S4[:].rearrange("p (t c) -> p t c", t=4),
                                in0=i0b, in1=i1b,
                                op=mybir.AluOpType.is_equal)
                            for k in range(4):
                                nc.tensor.matmul(
                                    segps[:, 12 * k:12 * k + 12],
                                    lhsT=S4[:, P * k:P * (k + 1)],
                                    rhs=tr12[:, 12 * k:12 * k + 12],
                                    start=True, stop=True,
                                    skip_group_check=True)
                            segsb = scp.tile([P, 48], BF16, tag="segsb",
                                             name="segsb")
                            nc.scalar.activation(
                                segsb[:], segps,
                                mybir.ActivationFunctionType.Copy)
                            eng = nc.scalar if bg % 2 else nc.sync
                            eng.dma_start(
                                out=blk_t[bg // 4][:, 48 * (bg % 4):
                                                   48 * (bg % 4) + 48],
                                in_=segsb[:])

            build_half(ablo, 0)
            run_bucket(0)
            build_half(abhi, NPAD // 2)
            run_bucket(1)
            run_bucket(2)
            run_bucket(3)

            loop_cm.__exit__(None, None, None)

    nc.finalize()
    return nc


def _prep_core(rows, cols, cdm, attr):
    """Bucket + row-sort + window-cut tile packing for one core's edges.

    Returns idxg [P, NBUCK*GI_PER_B*2*(NI/16)] i16, rowoff [P, TTOT] f32
    (row mod 128), cdm12 [P, TTOT*12] bf16 (zero-expanded over chunks),
    attrT [NBUCK, 1, TB*P] bf16, wbase [TTOT] i64.
    """
    idxg = np.zeros((NBUCK, GI_PER_B, 2, P, NI // 16), np.int16)
    roff = np.full((TTOT, P), P - 1, np.float32)
    cdm12 = np.zeros((TTOT, P, WC, 3), np.float32)
    attrT = np.zeros((NBUCK, 1, TB * P), ml_dtypes.bfloat16)
    wbase = np.zeros(TTOT, np.int64)
    bucket = (rows >= HALF).astype(np.int64) * 2 + (cols >= HALF).astype(np.int64)
    for b in range(NBUCK):
        sel = np.nonzero(bucket == b)[0]
        eb = len(sel)
        order = sel[np.argsort(rows[sel], kind="stable")]
        rs = rows[order]
        a_idx = np.zeros(TB * P, np.int16)
        b_idx = np.zeros(TB * P, np.int16)
        at = np.zeros(TB * P, np.float32)
        i = 0
        t = 0
        while i < eb:
            wb = (int(rs[i]) // P) * P
            hi = int(np.searchsorted(rs, wb + W, side="left"))
            j = min(i + P, hi, eb)
            n = j - i
            assert t < TB, f"bucket {b}: tile slots exhausted"
            tg = b * TB + t
            wbase[tg] = wb
            sl = order[i:j]
            a_idx[t * P:t * P + n] = (rows[sl] - (HALF if b >= 2 else 0)
                                      ).astype(np.int16)
            b_idx[t * P:t * P + n] = (cols[sl] - (HALF if b % 2 else 0)
                                      ).astype(np.int16)
            roff[tg, :n] = (rs[i:j] % P).astype(np.float32)
            ch = (rs[i:j] - wb) // P                 # chunk 0..WC-1
            cdm12[tg, np.arange(n), ch] = cdm[sl]
            at[t * P:t * P + n] = attr[sl]
            i = j
            t += 1
        attrT[b, 0, :] = at.astype(ml_dtypes.bfloat16)
        for gi in range(GI_PER_B):
            seg = slice(gi * NI, (gi + 1) * NI)
            idxg[b, gi, 0] = _wrap_idx(a_idx[seg])
            idxg[b, gi, 1] = _wrap_idx(b_idx[seg])
    idxg_p = np.ascontiguousarray(
        idxg.transpose(3, 0, 1, 2, 4).reshape(P, -1))
    cdm12_p = np.ascontiguousarray(
        cdm12.reshape(TTOT, P, 12).transpose(1, 0, 2).reshape(P, TTOT * 12)
    ).astype(ml_dtypes.bfloat16)
    return (idxg_p, roff.T.astype(np.float16).copy(), cdm12_p, attrT,
            wbase)


def prep_in_maps(h, rows, cols, cdm, attr, W1, b1, W2, b2, W3):
    bf = ml_dtypes.bfloat16
    hTp = np.zeros((H, NPAD), np.float32)
    hTp[:, :N] = h.T
    hT = np.ascontiguousarray(hTp).astype(bf)
    w1abT = np.ascontiguousarray(
        np.concatenate([W1[:, :H].T, W1[:, H:2 * H].T], axis=1)).astype(bf)
    w1c = np.ascontiguousarray(W1[:, 2 * H][None, :]).astype(bf)
    w2T = np.ascontiguousarray(W2.T).astype(bf)
    w3x12 = np.ascontiguousarray(
        np.repeat(W3.T / 100.0, 12, axis=1)).astype(bf)
    b1c = np.ascontiguousarray(b1[:, None]).astype(np.float32)
    b2c = np.ascontiguousarray(b2[:, None]).astype(np.float32)

    in_maps = []
    wbases = []
    for c in range(NCORES):
        sl = slice(c * ECORE, (c + 1) * ECORE)
        idxg, roffc, cdm12c, attrTc, wbasec = _prep_core(
            rows[sl], cols[sl], cdm[sl], attr[sl])
        in_maps.append({
            "hT": hT, "w1abT": w1abT, "w1c": w1c, "w2T": w2T, "w3x12": w3x12,
            "b1c": b1c, "b2c": b2c,
            "idxg": idxg, "roff": roffc, "cdm12": cdm12c, "attrT": attrTc,
        })
        wbases.append(wbasec)
    return in_maps, wbases


def kernel(h, coord, edge_index, coord_diff, edge_attr, node_mask, edge_mask,
           W1, b1, W2, b2, W3):
    h = np.asarray(h, np.float32)
    coord = np.asarray(coord, np.float32)
    edge_index = np.asarray(edge_index)
    coord_diff = np.asarray(coord_diff, np.float32)
    edge_attr = np.asarray(edge_attr, np.float32)
    node_mask = np.asarray(node_mask, np.float32)
    edge_mask = np.asarray(edge_mask, np.float32)
    W1 = np.asarray(W1, np.float32)
    b1 = np.asarray(b1, np.float32)
    W2 = np.asarray(W2, np.float32)
    b2 = np.asarray(b2, np.float32)
    W3 = np.asarray(W3, np.float32)

    rows = edge_index[0].astype(np.int32)
    cols = edge_index[1].astype(np.int32)
    cdm = coord_diff * edge_mask  # fold edge mask

    if "nc" not in _nc_cache:
        _nc_cache["nc"] = _build_program()
    nc = _nc_cache["nc"]

    in_maps, wbases = prep_in_maps(h, rows, cols, cdm, edge_attr[:, 0],
                                   W1, b1, W2, b2, W3)
    res = run_bass_kernel_spmd(nc, in_maps, list(range(NCORES))).results

    # combine: blk [NBLK, P, 48]; tile tg=4*bg+k occupies cols 12k:12k+12
    # holding [WC, 3] per partition with node = wbase[tg] + 128*j + p
    NTPAD = (N + W) // P + 1
    aggt = np.zeros((NTPAD, P, 3), np.float64)
    for c in range(NCORES):
        blk = np.asarray(res[c]["blk"], np.float64)     # [NBLK/4, P, 192]
        blk = blk.reshape(NBLK // 4, P, 4, 4, WC, 3)    # r(blk), k(tile), j
        blk = blk.transpose(0, 2, 3, 4, 1, 5).reshape(TTOT, WC, P, 3)
        tidx = wbases[c][:, None] // P + np.arange(WC)[None, :]
        np.add.at(aggt, tidx, blk)
    agg = aggt.reshape(NTPAD * P, 3)[:N].astype(np.float32)
    return (coord + agg) * node_mask
